# revision 1
# baseline (speedup 1.0000x reference)
"""DIFF cross-attention kernel for 8 Trainium2 NeuronCores.

Sharding: tensor-parallel over heads x data-parallel over batch.
Core r handles batch b = r//4 and head group g = r%4 (4 of 16 heads).

Device math (per core, everything in "transposed" channel-major layout):
  q1T/q2T [hd, Nq], k1T/k2T [hd, Nk]  via projections of query.T / key.T
  v1ext   [Nk, 65] per head (64 v-channels + ones column for softmax sums)
  scoresT [keys, q] = k1T_h.T @ q1T_h    (PE, K=64)
  eT = exp(scoresT * 1/8)                (ACT, fused scale, no max-sub:
                                          scores are provably < ~27)
  u_ext [65, q] = v1ext.T @ eT           (PE, rows 0-63 = unnorm out,
                                          row 64 = softmax denominator)
  xT[ch, q] = u1/l1 - lambda*u2/l2       (DVE + gpsimd partition_broadcast)
  ss[q] = sum_ch x^2                     (DVE square + PE ones-matmul)
  y_pT [out, q] = (x*norm_w)T @ proj_colsT    (partial proj, row-parallel)
  ONE ReduceScatter over each 4-core batch group of [y_pT | ss] packed
  as 4 blocks of 257 rows; then out = rsqrt(ss/1024+eps)*y + proj_b.

All matmul operands are float32r (fp32 bits, relaxed-precision matmul,
full PE rate at N>=256). The attention phase is ACT(exp)-bound, so the
second half of the k projections is emitted interleaved into the first
head's attention loop as PE filler work. Host returns
out[b][t, c] = y_out[r][c', t].
"""

import numpy as np

B = 2
NQ = 1024
NK = 2048
DIM = 1024
H = 16
HD = 64
NH = 4            # heads per core
G = 4             # cores per batch group
SCALE = 0.125
LAMBDA_INIT = 0.1
EPS = 1e-6
P = 128
DC = DIM // P     # 8 contraction chunks
KT = NK // P      # 16 key tiles
GROUPS = [[0, 1, 2, 3], [4, 5, 6, 7]]


def _build(stop_after="full", loop_n=0):
    import concourse.bass as bass
    import concourse.tile as tile
    import concourse.mybir as mybir
    from concourse import bacc

    f32 = mybir.dt.float32
    f32r = mybir.dt.float32r
    AF = mybir.ActivationFunctionType

    nc = bacc.Bacc("TRN2", target_bir_lowering=False, debug=False, num_devices=8)

    qT_d = nc.dram_tensor("qT", [DIM, NQ], f32r, kind="ExternalInput")
    kT_d = nc.dram_tensor("kT", [DIM, NK], f32r, kind="ExternalInput")
    wq1_d = nc.dram_tensor("wq1", [DIM, 256], f32r, kind="ExternalInput")
    wq2_d = nc.dram_tensor("wq2", [DIM, 256], f32r, kind="ExternalInput")
    wk1_d = nc.dram_tensor("wk1", [DIM, 256], f32r, kind="ExternalInput")
    wv1_d = nc.dram_tensor("wv1", [DIM, 256], f32r, kind="ExternalInput")
    wk2_d = nc.dram_tensor("wk2", [DIM, 256], f32r, kind="ExternalInput")
    wpT_d = nc.dram_tensor("wpT", [256, DIM], f32r, kind="ExternalInput")
    nw_d = nc.dram_tensor("nw", [P, 2], f32, kind="ExternalInput")
    pb_d = nc.dram_tensor("pb", [P, 2], f32, kind="ExternalInput")
    lamn_d = nc.dram_tensor("lamn", [1, NH], f32, kind="ExternalInput")
    y_out_d = nc.dram_tensor("y_out", [256, NQ], f32, kind="ExternalOutput")

    def _trace(tc):
        with (
            tc.tile_pool(name="res", bufs=1) as res,
            tc.tile_pool(name="dram", bufs=1, space="DRAM") as dram,
        ):
            # ---- resident tensors; big loads split per chunk to spread
            #      across DMA queues and unlock early compute ----
            kt_sb = res.tile([P, DC, NK], f32r)
            wpT = res.tile([P, 2, DIM], f32r)
            nw = res.tile([P, 2], f32)
            nc.sync.dma_start(nw[:], nw_d[:])
            pb = res.tile([P, 2], f32)
            nc.sync.dma_start(pb[:], pb_d[:])
            lamn = res.tile([1, NH], f32)
            nc.sync.dma_start(lamn[:], lamn_d[:])
            ones_f = res.tile([P, KT], f32)
            nc.vector.memset(ones_f[:], 1.0)
            ones_l = res.tile([P, 1], f32r)
            nc.vector.tensor_copy(ones_l[:], ones_f[:, 0:1])
            eps_t = res.tile([1, 1], f32)
            nc.vector.memset(eps_t[:], EPS)

            q1T = res.tile([P, 2, NQ], f32r)
            q2T = res.tile([P, 2, NQ], f32r)
            k1T = res.tile([P, 2, NK], f32r)
            k2T = res.tile([P, 2, NK], f32r)
            v1e = res.tile([P, NH, KT, 65], f32r)
            for h in range(NH):
                nc.vector.tensor_copy(v1e[:, h, :, 64:65],
                                      ones_f[:].unsqueeze(-1))
            xT = res.tile([P, 2, NQ], f32r)

            y_bounce1 = dram.tile([G * P, NQ], f32)
            y_bounce2 = dram.tile([G * 129, NQ], f32)
            y_red1 = dram.tile([P, NQ], f32)
            y_red2 = dram.tile([129, NQ], f32)

            wk1 = res.tile([P, DC, 256], f32r, name="wk1s")
            wv1 = res.tile([P, DC, 256], f32r, name="wv1s")
            wk2 = res.tile([P, DC, 256], f32r, name="wk2s")

            with (
                tc.tile_pool(name="ps_acc", bufs=8, space="PSUM") as ps_acc,
                tc.tile_pool(name="qstr", bufs=3) as qstr,
            ):
                # ---- phase A1: q projections (stream qT + wq, d-outer) ----
                psq = [ps_acc.tile([P, 512], f32, tag="acc", name=f"psq{i}")
                       for i in range(8)]
                for d in range(DC):
                    qt_c = qstr.tile([P, NQ], f32r, tag="qt")
                    nc.sync.dma_start(qt_c[:], qT_d[d * P:(d + 1) * P, :])
                    wq1_c = qstr.tile([P, 256], f32r, tag="wq1c")
                    nc.sync.dma_start(wq1_c[:], wq1_d[d * P:(d + 1) * P, :])
                    wq2_c = qstr.tile([P, 256], f32r, tag="wq2c")
                    nc.sync.dma_start(wq2_c[:], wq2_d[d * P:(d + 1) * P, :])
                    for pj, wc in ((0, wq1_c), (1, wq2_c)):
                        for m in range(2):
                            for qc in range(2):
                                nc.tensor.matmul(
                                    psq[pj * 4 + m * 2 + qc][:],
                                    wc[:, m * P:(m + 1) * P],
                                    qt_c[:, qc * 512:(qc + 1) * 512],
                                    start=(d == 0), stop=(d == DC - 1),
                                )
                for pj, dst in ((0, q1T), (1, q2T)):
                    for m in range(2):
                        for qc in range(2):
                            nc.vector.tensor_copy(
                                dst[:, m, qc * 512:(qc + 1) * 512],
                                psq[pj * 4 + m * 2 + qc][:],
                            )

                for d in range(DC):
                    nc.sync.dma_start(kt_sb[:, d, :],
                                      kT_d[d * P:(d + 1) * P, :])
                for t_, d_ in ((wk1, wk1_d), (wk2, wk2_d), (wv1, wv1_d)):
                    for hh in range(2):
                        nc.sync.dma_start(
                            t_[:, hh * 4:(hh + 1) * 4, :],
                            d_[hh * 512:(hh + 1) * 512, :].rearrange(
                                "(o p) n -> p o n", p=P))
                if stop_after == "loads":
                    nc.sync.dma_start(y_out_d[0:P, 0:DC],
                                      kt_sb[:, :, 0].bitcast(f32))
                    return

                if stop_after == "qproj":
                    nc.sync.dma_start(y_out_d[0:P, :], q1T[:, 0, :].bitcast(f32))
                    return

                # ---- phase A2: k1/k2 projections, m=0 only (d-inner) ----
                def kproj_items(m, pool):
                    """Yield callables emitting one instruction each for the
                    k-projections of hd-slice m (both kv1-k and kv2-k)."""
                    for wsrc, dst in ((wk1, k1T), (wk2, k2T)):
                        for kc in range(4):
                            pst = pool.tile([P, 512], f32, tag="acc",
                                            name=f"kp{m}_{kc}")
                            for d in range(DC):
                                yield lambda pst=pst, wsrc=wsrc, d=d, kc=kc, m=m: \
                                    nc.tensor.matmul(
                                        pst[:],
                                        wsrc[:, d, m * P:(m + 1) * P],
                                        kt_sb[:, d, kc * 512:(kc + 1) * 512],
                                        start=(d == 0), stop=(d == DC - 1),
                                    )
                            yield lambda pst=pst, dst=dst, kc=kc, m=m: \
                                nc.vector.tensor_copy(
                                    dst[:, m, kc * 512:(kc + 1) * 512], pst[:])

                for item in kproj_items(0, ps_acc):
                    item()

                if stop_after == "kproj0":
                    nc.sync.dma_start(y_out_d[0:P, :], k1T[:, 0, :NQ].bitcast(f32))
                    return

                # ---- phase A3: v1 (d-outer, one psum bank per kt,
                #      two passes of 8 kt: start=True clears has_written
                #      bank-wide, so accumulation groups can't share banks) ----
                for half in range(2):
                    psv = [ps_acc.tile([P, 512], f32, tag="acc",
                                       name=f"psv{half}_{i}") for i in range(8)]
                    for d in range(DC):
                        for i in range(8):
                            kt = half * 8 + i
                            nc.tensor.matmul(
                                psv[i][:, 0:256],
                                kt_sb[:, d, kt * P:(kt + 1) * P],
                                wv1[:, d, :],
                                start=(d == 0), stop=(d == DC - 1),
                            )
                    for i in range(8):
                        kt = half * 8 + i
                        for h in range(NH):
                            nc.vector.tensor_copy(
                                v1e[:, h, kt, 0:64],
                                psv[i][:, h * 64:h * 64 + 64],
                            )

            if stop_after == "proj":
                nc.sync.dma_start(y_out_d[0:P, :], k1T[:, 0, :NQ].bitcast(f32))
                return

            # ---- phase B: attention, with k-proj m=1 interleaved as PE
            #      filler inside head 0/1 (ACT-bound phase) ----
            xwp_cm = tc.tile_pool(name="xwp", bufs=2)
            x2p_cm = tc.tile_pool(name="x2p", bufs=2)
            xwp = xwp_cm.__enter__()
            x2p = x2p_cm.__enter__()
            xw = []

            def emit_xw(t):
                xw_t = xwp.tile([P, NQ], f32r, tag="xw", name=f"xw_{t}")
                nc.vector.tensor_scalar_mul(xw_t[:], xT[:, t, :],
                                            nw[:, t:t + 1])
                xw.append(xw_t)

            with (
                tc.tile_pool(name="ps_sc", bufs=2, space="PSUM") as ps_sc,
                tc.tile_pool(name="ps_u", bufs=2, space="PSUM") as ps_u,
                tc.tile_pool(name="ps_fil", bufs=2, space="PSUM") as ps_fil,
                tc.tile_pool(name="att", bufs=3) as att,
                tc.tile_pool(name="smal", bufs=2) as smal,
            ):
                for t in range(2):
                    nc.sync.dma_start(wpT[:, t, :],
                                      wpT_d[t * P:(t + 1) * P, :])

                filler = kproj_items(1, ps_fil)
                fill_done = False

                def emit_fill(k):
                    nonlocal fill_done
                    if fill_done:
                        return
                    for _ in range(k):
                        it = next(filler, None)
                        if it is None:
                            fill_done = True
                            return
                        it()

                for h in range(NH):
                    po = (h % 2) * 64
                    mi = h // 2
                    for qb in range(2):
                        qs = slice(qb * 512, (qb + 1) * 512)
                        u_ps = []
                        for br, ktp, qtp in ((0, k1T, q1T), (1, k2T, q2T)):
                            u = ps_u.tile([65, 512], f32, tag="u")
                            u_ps.append(u)
                            for kg in range(KT // 2):
                                sc = ps_sc.tile([P, 1024], f32, tag="sc")
                                for j in range(2):
                                    kt = kg * 2 + j
                                    nc.tensor.matmul(
                                        sc[:, j * 512:(j + 1) * 512],
                                        ktp[po:po + 64, mi, kt * P:(kt + 1) * P],
                                        qtp[po:po + 64, mi, qs],
                                        start=True, stop=True,
                                    )
                                e_t = att.tile([P, 1024], f32r, tag="e")
                                nc.scalar.activation(e_t[:], sc[:], AF.Exp,
                                                     scale=SCALE)
                                for j in range(2):
                                    kt = kg * 2 + j
                                    nc.tensor.matmul(
                                        u[:],
                                        v1e[:, h, kt, :],
                                        e_t[:, j * 512:(j + 1) * 512],
                                        start=(kt == 0), stop=(kt == KT - 1),
                                    )
                                emit_fill(3)
                        # combine branches: x = u1/l1 - lambda*u2/l2
                        rr1 = smal.tile([1, 512], f32, tag="rr")
                        nc.vector.reciprocal(rr1[:], u_ps[0][64:65, :])
                        rr2 = smal.tile([1, 512], f32, tag="rr")
                        nc.vector.reciprocal(rr2[:], u_ps[1][64:65, :])
                        nc.vector.tensor_scalar_mul(rr2[:], rr2[:],
                                                    lamn[0:1, h:h + 1])
                        rr1b = smal.tile([64, 512], f32, tag="rrb")
                        nc.gpsimd.partition_broadcast(rr1b[:], rr1[:])
                        rr2b = smal.tile([64, 512], f32, tag="rrb")
                        nc.gpsimd.partition_broadcast(rr2b[:], rr2[:])
                        t1 = smal.tile([64, 512], f32, tag="tt")
                        nc.vector.tensor_mul(t1[:], u_ps[0][0:64, :], rr1b[:])
                        t2 = smal.tile([64, 512], f32, tag="tt")
                        nc.vector.tensor_mul(t2[:], u_ps[1][0:64, :], rr2b[:])
                        nc.vector.tensor_add(xT[po:po + 64, mi, qs],
                                             t1[:], t2[:])
                        if h == 1 and qb == 1:
                            emit_xw(0)

            if stop_after == "attn":
                x2p_cm.__exit__(None, None, None)
                xwp_cm.__exit__(None, None, None)
                nc.sync.dma_start(y_out_d[0:P, :], xT[:, 0, :].bitcast(f32))
                return

            # ---- phase C: tail (split ReduceScatter: even out-tiles first,
            #      RS1 overlaps the odd-tile projection + ss work) ----
            emit_xw(1)
            with (
                tc.tile_pool(name="ps_ss", bufs=1, space="PSUM") as ps_ss,
                tc.tile_pool(name="ps_yp", bufs=4, space="PSUM") as ps_yp,
                tc.tile_pool(name="ypp", bufs=2) as ypp,
            ):
                def yp_tile(m, dst, row0):
                    for qc in range(2):
                        yp = ps_yp.tile([P, 512], f32, tag="yp")
                        for t in range(2):
                            nc.tensor.matmul(
                                yp[:],
                                wpT[:, t, m * P:(m + 1) * P],
                                xw[t][:, qc * 512:(qc + 1) * 512],
                                start=(t == 0), stop=(t == 1),
                            )
                        yp_sb = ypp.tile([P, 512], f32, tag="ypsb")
                        nc.vector.tensor_copy(yp_sb[:], yp[:])
                        nc.sync.dma_start(
                            dst[row0:row0 + P, qc * 512:(qc + 1) * 512],
                            yp_sb[:],
                        )

                for g in range(G):
                    yp_tile(2 * g, y_bounce1, g * P)

                if stop_after != "precc":
                    nc.gpsimd.collective_compute(
                        "ReduceScatter",
                        mybir.AluOpType.add,
                        replica_groups=GROUPS,
                        ins=[y_bounce1.opt()],
                        outs=[y_red1.opt()],
                    )

                ss_ps = ps_ss.tile([1, NQ], f32)
                for t in range(2):
                    for qc in range(2):
                        x2c = x2p.tile([P, 512], f32r, tag="x2")
                        nc.vector.tensor_mul(
                            x2c[:], xT[:, t, qc * 512:(qc + 1) * 512],
                            xT[:, t, qc * 512:(qc + 1) * 512])
                        nc.tensor.matmul(
                            ss_ps[0:1, qc * 512:(qc + 1) * 512],
                            ones_l[:],
                            x2c[:],
                            start=(t == 0), stop=(t == 1),
                        )
                for g in range(G):
                    yp_tile(2 * g + 1, y_bounce2, g * 129)
                ss_sb = ypp.tile([1, NQ], f32, tag="sssb")
                nc.vector.tensor_copy(ss_sb[:], ss_ps[:])
                for gb in range(G):
                    nc.sync.dma_start(
                        y_bounce2[gb * 129 + 128:gb * 129 + 129, :],
                        ss_sb[:])

            # phase-C psum/sbuf pools closed (LIFO) before the final post work
            x2p_cm.__exit__(None, None, None)
            xwp_cm.__exit__(None, None, None)

            if stop_after == "precc":
                return

            nc.gpsimd.collective_compute(
                "ReduceScatter",
                mybir.AluOpType.add,
                replica_groups=GROUPS,
                ins=[y_bounce2.opt()],
                outs=[y_red2.opt()],
            )

            if True:
                with tc.tile_pool(name="post", bufs=1) as post:
                    ss_row = post.tile([1, NQ], f32, tag="ssrow")
                    nc.sync.dma_start(ss_row[:], y_red2[128:129, :])
                    s_row = post.tile([1, NQ], f32, tag="srow")
                    nc.scalar.activation(s_row[:], ss_row[:], AF.Sqrt,
                                         bias=eps_t[0:1, 0:1], scale=1.0 / DIM)
                    nc.vector.reciprocal(s_row[:], s_row[:])
                    s_b = post.tile([P, NQ], f32, tag="sb")
                    nc.gpsimd.partition_broadcast(s_b[:], s_row[:])
                    for t, src_red in ((0, y_red1), (1, y_red2)):
                        yred_t = post.tile([P, NQ], f32, tag="yred")
                        nc.sync.dma_start(yred_t[:], src_red[0:P, :])
                        nc.vector.tensor_mul(yred_t[:], yred_t[:], s_b[:])
                        nc.vector.tensor_scalar_add(yred_t[:], yred_t[:],
                                                    pb[:, t:t + 1])
                        nc.sync.dma_start(y_out_d[t * P:(t + 1) * P, :],
                                          yred_t[:])

    with tile.TileContext(nc) as tc:
        if loop_n:
            with tc.For_i(0, loop_n, 1):
                _trace(tc)
        else:
            _trace(tc)
    nc.compile()
    return nc


_CACHE = {}


def _get_nc():
    if "nc" not in _CACHE:
        _CACHE["nc"] = _build()
    return _CACHE["nc"]


def _shard_inputs(inputs):
    q = np.asarray(inputs["query"], np.float32)
    k = np.asarray(inputs["key"], np.float32)
    q1_w = np.asarray(inputs["q1_w"], np.float32)
    q2_w = np.asarray(inputs["q2_w"], np.float32)
    kv1_w = np.asarray(inputs["kv1_w"], np.float32)
    kv2_w = np.asarray(inputs["kv2_w"], np.float32)
    proj_w = np.asarray(inputs["proj_w"], np.float32)
    proj_b = np.asarray(inputs["proj_b"], np.float32)
    norm_w = np.asarray(inputs["norm_w"], np.float32)
    lam1 = np.asarray(inputs["lambda_1"], np.float32).reshape(H)
    lam2 = np.asarray(inputs["lambda_2"], np.float32).reshape(H)
    lam_full = lam1 - lam2 + LAMBDA_INIT

    c = np.ascontiguousarray
    in_maps = []
    for r in range(8):
        b, g = r // G, r % G
        rows = slice(g * 256, (g + 1) * 256)
        vrows = slice(DIM + g * 256, DIM + (g + 1) * 256)
        in_maps.append({
            "qT": c(q[b].T),
            "kT": c(k[b].T),
            "wq1": c(q1_w[rows].T),
            "wq2": c(q2_w[rows].T),
            "wk1": c(kv1_w[rows].T),
            "wv1": c(kv1_w[vrows].T),
            "wk2": c(kv2_w[rows].T),
            "wpT": c(proj_w[:, rows].T),
            "nw": c(norm_w[rows].reshape(2, P).T),
            "pb": c(proj_b[rows].reshape(2, P).T),
            "lamn": c(-lam_full[g * NH:(g + 1) * NH].reshape(1, NH)),
        })
    return in_maps


def kernel(**inputs):
    from concourse.bass_utils import run_bass_kernel_spmd

    nc = _get_nc()
    in_maps = _shard_inputs(inputs)
    res = run_bass_kernel_spmd(nc, in_maps, core_ids=list(range(8)))
    out = np.empty((B, NQ, DIM), np.float32)
    for r in range(8):
        b, g = r // G, r % G
        out[b, :, g * 256:(g + 1) * 256] = res.results[r]["y_out"].T
    return out



# revision 5
# speedup vs baseline: 1.7180x; 1.7180x over previous
"""DIFF cross-attention kernel for 8 Trainium2 NeuronCores.

Sharding: tensor-parallel over heads x data-parallel over batch.
Core r handles batch b = r//4 and head group g = r%4 (4 of 16 heads).

Device math (per core, everything in "transposed" channel-major layout):
  q1T/q2T [hd, Nq], k1T/k2T [hd, Nk]  via projections of query.T / key.T
  v1ext   [Nk, 65] per head (64 v-channels + ones column for softmax sums)
  scoresT [keys, q] = k1T_h.T @ q1T_h    (PE, K=64)
  eT = exp(scoresT * 1/8)                (ACT, fused scale, no max-sub:
                                          scores are provably < ~27)
  u_ext [65, q] = v1ext.T @ eT           (PE, rows 0-63 = unnorm out,
                                          row 64 = softmax denominator)
  xT[ch, q] = u1/l1 - lambda*u2/l2       (DVE + gpsimd partition_broadcast)
  ss[q] = sum_ch x^2                     (DVE square + PE ones-matmul)
  y_pT [out, q] = (x*norm_w)T @ proj_colsT    (partial proj, row-parallel)
  ONE ReduceScatter over each 4-core batch group of [y_pT | ss] packed
  as 4 blocks of 257 rows; then out = rsqrt(ss/1024+eps)*y + proj_b.

All matmul operands are float32r (fp32 bits, relaxed-precision matmul,
full PE rate at N>=256). The attention phase is ACT(exp)-bound, so the
second half of the k projections is emitted interleaved into the first
head's attention loop as PE filler work. Host returns
out[b][t, c] = y_out[r][c', t].
"""

import numpy as np

B = 2
NQ = 1024
NK = 2048
DIM = 1024
H = 16
HD = 64
NH = 4            # heads per core
G = 4             # cores per batch group
SCALE = 0.125
LAMBDA_INIT = 0.1
EPS = 1e-6
P = 128
DC = DIM // P     # 8 contraction chunks
KT = NK // P      # 16 key tiles
GROUPS = [[0, 1, 2, 3], [4, 5, 6, 7]]


def _build(stop_after="full", loop_n=0):
    import concourse.bass as bass
    import concourse.tile as tile
    import concourse.mybir as mybir
    from concourse import bacc

    f32 = mybir.dt.float32
    f32r = mybir.dt.float32r
    bf16 = mybir.dt.bfloat16
    AF = mybir.ActivationFunctionType

    nc = bacc.Bacc("TRN2", target_bir_lowering=False, debug=False, num_devices=8)

    qT_d = nc.dram_tensor("qT", [DIM, NQ], bf16, kind="ExternalInput")
    kT_d = nc.dram_tensor("kT", [DIM, NK], bf16, kind="ExternalInput")
    wq1_d = nc.dram_tensor("wq1", [DIM, 256], bf16, kind="ExternalInput")
    wq2_d = nc.dram_tensor("wq2", [DIM, 256], bf16, kind="ExternalInput")
    wk1_d = nc.dram_tensor("wk1", [DIM, 256], bf16, kind="ExternalInput")
    wv1_d = nc.dram_tensor("wv1", [DIM, 256], bf16, kind="ExternalInput")
    wk2_d = nc.dram_tensor("wk2", [DIM, 256], bf16, kind="ExternalInput")
    wpT_d = nc.dram_tensor("wpT", [256, DIM], bf16, kind="ExternalInput")
    nw_d = nc.dram_tensor("nw", [P, 2], f32, kind="ExternalInput")
    pb_d = nc.dram_tensor("pb", [P, 2], f32, kind="ExternalInput")
    lamn_d = nc.dram_tensor("lamn", [1, NH], f32, kind="ExternalInput")
    y_out_d = nc.dram_tensor("y_out", [256, NQ], f32, kind="ExternalOutput")

    def _trace(tc):
        with (
            tc.tile_pool(name="res", bufs=1) as res,
            tc.tile_pool(name="dram", bufs=1, space="DRAM") as dram,
        ):
            # ---- resident tensors; big loads split per chunk to spread
            #      across DMA queues and unlock early compute ----
            kt_sb = res.tile([P, DC, NK], bf16)
            wpT = res.tile([P, 2, DIM], bf16)
            nw = res.tile([P, 2], f32)
            nc.sync.dma_start(nw[:], nw_d[:])
            pb = res.tile([P, 2], f32)
            nc.sync.dma_start(pb[:], pb_d[:])
            lamn = res.tile([1, NH], f32)
            nc.sync.dma_start(lamn[:], lamn_d[:])
            ones_f = res.tile([P, KT], f32)
            nc.vector.memset(ones_f[:], 1.0)
            ones_l = res.tile([P, 1], f32r)
            nc.vector.tensor_copy(ones_l[:], ones_f[:, 0:1])
            eps_t = res.tile([1, 1], f32)
            nc.vector.memset(eps_t[:], EPS)

            q1T = res.tile([P, 2, NQ], bf16)
            q2T = res.tile([P, 2, NQ], bf16)
            k1T = res.tile([P, 2, NK], bf16)
            k2T = res.tile([P, 2, NK], bf16)
            v1e = res.tile([P, NH, KT, 65], bf16)
            for h in range(NH):
                nc.vector.tensor_copy(v1e[:, h, :, 64:65],
                                      ones_f[:].unsqueeze(-1))
            xT = res.tile([P, 2, NQ], f32r)

            y_bounce1 = dram.tile([G * P, NQ], f32)
            y_bounce2 = dram.tile([G * 129, NQ], f32)
            y_red1 = dram.tile([P, NQ], f32)
            y_red2 = dram.tile([129, NQ], f32)

            wk1 = res.tile([P, DC, 256], bf16, name="wk1s")
            wv1 = res.tile([P, DC, 256], bf16, name="wv1s")
            wk2 = res.tile([P, DC, 256], bf16, name="wk2s")

            with (
                tc.tile_pool(name="ps_acc", bufs=8, space="PSUM") as ps_acc,
                tc.tile_pool(name="qstr", bufs=3) as qstr,
            ):
                # ---- phase A1: q projections (stream qT + wq, d-outer) ----
                psq = [ps_acc.tile([P, 512], f32, tag="acc", name=f"psq{i}")
                       for i in range(8)]
                for d in range(DC):
                    qt_c = qstr.tile([P, NQ], bf16, tag="qt")
                    nc.sync.dma_start(qt_c[:], qT_d[d * P:(d + 1) * P, :])
                    wq1_c = qstr.tile([P, 256], bf16, tag="wq1c")
                    nc.sync.dma_start(wq1_c[:], wq1_d[d * P:(d + 1) * P, :])
                    wq2_c = qstr.tile([P, 256], bf16, tag="wq2c")
                    nc.sync.dma_start(wq2_c[:], wq2_d[d * P:(d + 1) * P, :])
                    for pj, wc in ((0, wq1_c), (1, wq2_c)):
                        for m in range(2):
                            for qc in range(2):
                                nc.tensor.matmul(
                                    psq[pj * 4 + m * 2 + qc][:],
                                    wc[:, m * P:(m + 1) * P],
                                    qt_c[:, qc * 512:(qc + 1) * 512],
                                    start=(d == 0), stop=(d == DC - 1),
                                )
                for pj, dst in ((0, q1T), (1, q2T)):
                    for m in range(2):
                        for qc in range(2):
                            nc.vector.tensor_copy(
                                dst[:, m, qc * 512:(qc + 1) * 512],
                                psq[pj * 4 + m * 2 + qc][:],
                            )

                for d in range(DC):
                    nc.sync.dma_start(kt_sb[:, d, :],
                                      kT_d[d * P:(d + 1) * P, :])
                for t_, d_ in ((wk1, wk1_d), (wk2, wk2_d), (wv1, wv1_d)):
                    for hh in range(2):
                        nc.sync.dma_start(
                            t_[:, hh * 4:(hh + 1) * 4, :],
                            d_[hh * 512:(hh + 1) * 512, :].rearrange(
                                "(o p) n -> p o n", p=P))
                if stop_after == "loads":
                    nc.sync.dma_start(y_out_d[0:P, 0:DC],
                                      kt_sb[:, :, 0].bitcast(f32))
                    return

                if stop_after == "qproj":
                    nc.sync.dma_start(y_out_d[0:P, :], q1T[:, 0, :].bitcast(f32))
                    return

                # ---- phase A2: k1/k2 projections, m=0 only (d-inner) ----
                def kproj_items(m, pool):
                    """Yield callables emitting one instruction each for the
                    k-projections of hd-slice m (both kv1-k and kv2-k)."""
                    for wsrc, dst in ((wk1, k1T), (wk2, k2T)):
                        for kc in range(4):
                            pst = pool.tile([P, 512], f32, tag="acc",
                                            name=f"kp{m}_{kc}")
                            for d in range(DC):
                                yield lambda pst=pst, wsrc=wsrc, d=d, kc=kc, m=m: \
                                    nc.tensor.matmul(
                                        pst[:],
                                        wsrc[:, d, m * P:(m + 1) * P],
                                        kt_sb[:, d, kc * 512:(kc + 1) * 512],
                                        start=(d == 0), stop=(d == DC - 1),
                                    )
                            yield lambda pst=pst, dst=dst, kc=kc, m=m: \
                                nc.vector.tensor_copy(
                                    dst[:, m, kc * 512:(kc + 1) * 512], pst[:])

                for item in kproj_items(0, ps_acc):
                    item()

                if stop_after == "kproj0":
                    nc.sync.dma_start(y_out_d[0:P, :], k1T[:, 0, :NQ].bitcast(f32))
                    return

                # ---- phase A3: v1 (d-outer, one psum bank per kt,
                #      two passes of 8 kt: start=True clears has_written
                #      bank-wide, so accumulation groups can't share banks) ----
                for half in range(2):
                    psv = [ps_acc.tile([P, 512], f32, tag="acc",
                                       name=f"psv{half}_{i}") for i in range(8)]
                    for d in range(DC):
                        for i in range(8):
                            kt = half * 8 + i
                            nc.tensor.matmul(
                                psv[i][:, 0:256],
                                kt_sb[:, d, kt * P:(kt + 1) * P],
                                wv1[:, d, :],
                                start=(d == 0), stop=(d == DC - 1),
                            )
                    for i in range(8):
                        kt = half * 8 + i
                        for h in range(NH):
                            nc.vector.tensor_copy(
                                v1e[:, h, kt, 0:64],
                                psv[i][:, h * 64:h * 64 + 64],
                            )

            if stop_after == "proj":
                nc.sync.dma_start(y_out_d[0:P, :], k1T[:, 0, :NQ].bitcast(f32))
                return

            # ---- phase B: attention, with k-proj m=1 interleaved as PE
            #      filler inside head 0/1 (ACT-bound phase) ----
            xwp_cm = tc.tile_pool(name="xwp", bufs=2)
            x2p_cm = tc.tile_pool(name="x2p", bufs=2)
            xwp = xwp_cm.__enter__()
            x2p = x2p_cm.__enter__()
            xw = []

            def emit_xw(t):
                xw_t = xwp.tile([P, NQ], bf16, tag="xw", name=f"xw_{t}")
                nc.vector.tensor_scalar_mul(xw_t[:], xT[:, t, :],
                                            nw[:, t:t + 1])
                xw.append(xw_t)

            with (
                tc.tile_pool(name="ps_sc", bufs=2, space="PSUM") as ps_sc,
                tc.tile_pool(name="ps_u", bufs=2, space="PSUM") as ps_u,
                tc.tile_pool(name="ps_fil", bufs=2, space="PSUM") as ps_fil,
                tc.tile_pool(name="att", bufs=3) as att,
                tc.tile_pool(name="smal", bufs=2) as smal,
            ):
                for t in range(2):
                    nc.sync.dma_start(wpT[:, t, :],
                                      wpT_d[t * P:(t + 1) * P, :])

                filler = kproj_items(1, ps_fil)
                fill_done = False

                def emit_fill(k):
                    nonlocal fill_done
                    if fill_done:
                        return
                    for _ in range(k):
                        it = next(filler, None)
                        if it is None:
                            fill_done = True
                            return
                        it()

                for h in range(NH):
                    po = (h % 2) * 64
                    mi = h // 2
                    for qb in range(2):
                        qs = slice(qb * 512, (qb + 1) * 512)
                        u_ps = []
                        for br, ktp, qtp in ((0, k1T, q1T), (1, k2T, q2T)):
                            u = ps_u.tile([65, 512], f32, tag="u")
                            u_ps.append(u)
                            for kg in range(KT // 2):
                                sc = ps_sc.tile([P, 1024], f32, tag="sc")
                                for j in range(2):
                                    kt = kg * 2 + j
                                    nc.tensor.matmul(
                                        sc[:, j * 512:(j + 1) * 512],
                                        ktp[po:po + 64, mi, kt * P:(kt + 1) * P],
                                        qtp[po:po + 64, mi, qs],
                                        start=True, stop=True,
                                    )
                                e_t = att.tile([P, 1024], bf16, tag="e")
                                nc.scalar.activation(e_t[:], sc[:], AF.Exp,
                                                     scale=SCALE)
                                for j in range(2):
                                    kt = kg * 2 + j
                                    nc.tensor.matmul(
                                        u[:],
                                        v1e[:, h, kt, :],
                                        e_t[:, j * 512:(j + 1) * 512],
                                        start=(kt == 0), stop=(kt == KT - 1),
                                    )
                                emit_fill(3)
                        # combine branches: x = u1/l1 - lambda*u2/l2
                        rr1 = smal.tile([1, 512], f32, tag="rr")
                        nc.vector.reciprocal(rr1[:], u_ps[0][64:65, :])
                        rr2 = smal.tile([1, 512], f32, tag="rr")
                        nc.vector.reciprocal(rr2[:], u_ps[1][64:65, :])
                        nc.vector.tensor_scalar_mul(rr2[:], rr2[:],
                                                    lamn[0:1, h:h + 1])
                        rr1b = smal.tile([64, 512], f32, tag="rrb")
                        nc.gpsimd.partition_broadcast(rr1b[:], rr1[:])
                        rr2b = smal.tile([64, 512], f32, tag="rrb")
                        nc.gpsimd.partition_broadcast(rr2b[:], rr2[:])
                        t1 = smal.tile([64, 512], f32, tag="tt")
                        nc.vector.tensor_mul(t1[:], u_ps[0][0:64, :], rr1b[:])
                        t2 = smal.tile([64, 512], f32, tag="tt")
                        nc.vector.tensor_mul(t2[:], u_ps[1][0:64, :], rr2b[:])
                        nc.vector.tensor_add(xT[po:po + 64, mi, qs],
                                             t1[:], t2[:])
                        if h == 1 and qb == 1:
                            emit_xw(0)

            if stop_after == "attn":
                x2p_cm.__exit__(None, None, None)
                xwp_cm.__exit__(None, None, None)
                nc.sync.dma_start(y_out_d[0:P, :], xT[:, 0, :].bitcast(f32))
                return

            # ---- phase C: tail (split ReduceScatter: even out-tiles first,
            #      RS1 overlaps the odd-tile projection + ss work) ----
            emit_xw(1)
            with (
                tc.tile_pool(name="ps_ss", bufs=1, space="PSUM") as ps_ss,
                tc.tile_pool(name="ps_yp", bufs=4, space="PSUM") as ps_yp,
                tc.tile_pool(name="ypp", bufs=2) as ypp,
            ):
                def yp_tile(m, dst, row0):
                    for qc in range(2):
                        yp = ps_yp.tile([P, 512], f32, tag="yp")
                        for t in range(2):
                            nc.tensor.matmul(
                                yp[:],
                                wpT[:, t, m * P:(m + 1) * P],
                                xw[t][:, qc * 512:(qc + 1) * 512],
                                start=(t == 0), stop=(t == 1),
                            )
                        yp_sb = ypp.tile([P, 512], f32, tag="ypsb")
                        nc.vector.tensor_copy(yp_sb[:], yp[:])
                        nc.sync.dma_start(
                            dst[row0:row0 + P, qc * 512:(qc + 1) * 512],
                            yp_sb[:],
                        )

                for g in range(G):
                    yp_tile(2 * g, y_bounce1, g * P)

                if stop_after != "precc":
                    nc.gpsimd.collective_compute(
                        "ReduceScatter",
                        mybir.AluOpType.add,
                        replica_groups=GROUPS,
                        ins=[y_bounce1.opt()],
                        outs=[y_red1.opt()],
                    )

                ss_ps = ps_ss.tile([1, NQ], f32)
                for t in range(2):
                    for qc in range(2):
                        x2c = x2p.tile([P, 512], f32r, tag="x2")
                        nc.vector.tensor_mul(
                            x2c[:], xT[:, t, qc * 512:(qc + 1) * 512],
                            xT[:, t, qc * 512:(qc + 1) * 512])
                        nc.tensor.matmul(
                            ss_ps[0:1, qc * 512:(qc + 1) * 512],
                            ones_l[:],
                            x2c[:],
                            start=(t == 0), stop=(t == 1),
                        )
                for g in range(G):
                    yp_tile(2 * g + 1, y_bounce2, g * 129)
                ss_sb = ypp.tile([1, NQ], f32, tag="sssb")
                nc.vector.tensor_copy(ss_sb[:], ss_ps[:])
                for gb in range(G):
                    nc.sync.dma_start(
                        y_bounce2[gb * 129 + 128:gb * 129 + 129, :],
                        ss_sb[:])

            # phase-C psum/sbuf pools closed (LIFO) before the final post work
            x2p_cm.__exit__(None, None, None)
            xwp_cm.__exit__(None, None, None)

            if stop_after == "precc":
                return

            nc.gpsimd.collective_compute(
                "ReduceScatter",
                mybir.AluOpType.add,
                replica_groups=GROUPS,
                ins=[y_bounce2.opt()],
                outs=[y_red2.opt()],
            )

            if True:
                with tc.tile_pool(name="post", bufs=1) as post:
                    ss_row = post.tile([1, NQ], f32, tag="ssrow")
                    nc.sync.dma_start(ss_row[:], y_red2[128:129, :])
                    s_row = post.tile([1, NQ], f32, tag="srow")
                    nc.scalar.activation(s_row[:], ss_row[:], AF.Sqrt,
                                         bias=eps_t[0:1, 0:1], scale=1.0 / DIM)
                    nc.vector.reciprocal(s_row[:], s_row[:])
                    s_b = post.tile([P, NQ], f32, tag="sb")
                    nc.gpsimd.partition_broadcast(s_b[:], s_row[:])
                    for t, src_red in ((0, y_red1), (1, y_red2)):
                        yred_t = post.tile([P, NQ], f32, tag="yred")
                        nc.sync.dma_start(yred_t[:], src_red[0:P, :])
                        nc.vector.tensor_mul(yred_t[:], yred_t[:], s_b[:])
                        nc.vector.tensor_scalar_add(yred_t[:], yred_t[:],
                                                    pb[:, t:t + 1])
                        nc.sync.dma_start(y_out_d[t * P:(t + 1) * P, :],
                                          yred_t[:])

    with tile.TileContext(nc) as tc:
        if loop_n:
            with tc.For_i(0, loop_n, 1):
                _trace(tc)
        else:
            _trace(tc)
    nc.compile()
    return nc


_CACHE = {}


def _get_nc():
    if "nc" not in _CACHE:
        _CACHE["nc"] = _build()
    return _CACHE["nc"]


def _shard_inputs(inputs):
    q = np.asarray(inputs["query"], np.float32)
    k = np.asarray(inputs["key"], np.float32)
    q1_w = np.asarray(inputs["q1_w"], np.float32)
    q2_w = np.asarray(inputs["q2_w"], np.float32)
    kv1_w = np.asarray(inputs["kv1_w"], np.float32)
    kv2_w = np.asarray(inputs["kv2_w"], np.float32)
    proj_w = np.asarray(inputs["proj_w"], np.float32)
    proj_b = np.asarray(inputs["proj_b"], np.float32)
    norm_w = np.asarray(inputs["norm_w"], np.float32)
    lam1 = np.asarray(inputs["lambda_1"], np.float32).reshape(H)
    lam2 = np.asarray(inputs["lambda_2"], np.float32).reshape(H)
    lam_full = lam1 - lam2 + LAMBDA_INIT

    from ml_dtypes import bfloat16

    def c(a):
        return np.ascontiguousarray(a.astype(bfloat16))

    cf = np.ascontiguousarray
    in_maps = []
    for r in range(8):
        b, g = r // G, r % G
        rows = slice(g * 256, (g + 1) * 256)
        vrows = slice(DIM + g * 256, DIM + (g + 1) * 256)
        in_maps.append({
            "qT": c(q[b].T),
            "kT": c(k[b].T),
            "wq1": c(q1_w[rows].T),
            "wq2": c(q2_w[rows].T),
            "wk1": c(kv1_w[rows].T),
            "wv1": c(kv1_w[vrows].T),
            "wk2": c(kv2_w[rows].T),
            "wpT": c(proj_w[:, rows].T),
            "nw": cf(norm_w[rows].reshape(2, P).T),
            "pb": cf(proj_b[rows].reshape(2, P).T),
            "lamn": cf(-lam_full[g * NH:(g + 1) * NH].reshape(1, NH)),
        })
    return in_maps


def kernel(**inputs):
    from concourse.bass_utils import run_bass_kernel_spmd

    nc = _get_nc()
    in_maps = _shard_inputs(inputs)
    res = run_bass_kernel_spmd(nc, in_maps, core_ids=list(range(8)))
    out = np.empty((B, NQ, DIM), np.float32)
    for r in range(8):
        b, g = r // G, r % G
        out[b, :, g * 256:(g + 1) * 256] = res.results[r]["y_out"].T
    return out



# revision 15
# speedup vs baseline: 238147.0000x; 138616.0000x over previous
"""DIFF cross-attention kernel for 8 Trainium2 NeuronCores.

Sharding: tensor-parallel over heads x data-parallel over batch.
Core r handles batch b = r//4 and head group g = r%4 (4 of 16 heads).

Device math (per core, channel-major "transposed" layout):
  q1T/q2T [hd, Nq], k1T [hd, Nk] via projections of query.T / key.T
  v1ext   [Nk, 65] per head (64 v-channels + ones column for softmax sums)
  scoresT [keys, q] = k1T_h.T @ q1T_h    (PE; branch 2 runs in fp8e4 with
                                          DoubleRow packing at 2x PE rate,
                                          safe because branch 2 is scaled
                                          by lambda ~= 0.108)
  eT = exp(scoresT / 8)                  (ACT, fused scale; no max-sub:
                                          scores provably < ~27)
  u_ext [65, q] = v1ext.T @ eT           (PE; fp8 DoubleRow for branch 2)
  xT[ch, q] = u1/l1 - lambda*u2/l2       (DVE + gpsimd partition_broadcast)
  ss[q] = sum_ch x^2                     (DVE square + PE ones-matmul)
  y_pT [out, q] = (x*norm_w)T @ proj_colsT   (partial proj, row-parallel)

Schedule: attention runs q-block-outer (2 blocks of 512). Each q-block's
partial projection + ss are packed to bf16 and ReduceScattered over the
4-core batch group; qb0's collective and post-processing hide under
qb1's attention. All projections not needed for head 0 are emitted as
PE filler inside the ACT-bound attention loops. Inputs stream in bf16.
Host returns out[b][t, c] = y_out[r][c', t].
"""

import numpy as np

B = 2
NQ = 1024
NK = 2048
DIM = 1024
H = 16
HD = 64
NH = 4            # heads per core
G = 4             # cores per batch group
SCALE = 0.125
LAMBDA_INIT = 0.1
EPS = 1e-6
P = 128
DC = DIM // P     # 8 contraction chunks
KT = NK // P      # 16 key tiles
GROUPS = [[0, 1, 2, 3], [4, 5, 6, 7]]
USE_FP8 = True


def _build(stop_after="full", loop_n=0):
    import concourse.bass as bass
    import concourse.tile as tile
    import concourse.mybir as mybir
    from concourse import bacc

    f32 = mybir.dt.float32
    f32r = mybir.dt.float32r
    bf16 = mybir.dt.bfloat16
    f8 = mybir.dt.float8e4
    AF = mybir.ActivationFunctionType
    DR = mybir.MatmulPerfMode.DoubleRow

    nc = bacc.Bacc("TRN2", target_bir_lowering=False, debug=False, num_devices=8)

    qT_d = nc.dram_tensor("qT", [DIM, NQ], bf16, kind="ExternalInput")
    kT_d = nc.dram_tensor("kT", [DIM, NK], bf16, kind="ExternalInput")
    wq1_d = nc.dram_tensor("wq1", [DIM, 256], bf16, kind="ExternalInput")
    wq2_d = nc.dram_tensor("wq2", [DIM, 256], bf16, kind="ExternalInput")
    wk1_d = nc.dram_tensor("wk1", [DIM, 256], bf16, kind="ExternalInput")
    wv1_d = nc.dram_tensor("wv1", [DIM, 256], bf16, kind="ExternalInput")
    wk2_d = nc.dram_tensor("wk2", [DIM, 256], bf16, kind="ExternalInput")
    wpT_d = nc.dram_tensor("wpT", [256, DIM], bf16, kind="ExternalInput")
    nw_d = nc.dram_tensor("nw", [P, 2], f32, kind="ExternalInput")
    pb_d = nc.dram_tensor("pb", [P, 2], f32, kind="ExternalInput")
    lamn_d = nc.dram_tensor("lamn", [1, NH], f32, kind="ExternalInput")
    y_out_d = nc.dram_tensor("y_out", [256, NQ], f32, kind="ExternalOutput")

    def _trace(tc):
        with (
            tc.tile_pool(name="res", bufs=1) as res,
            tc.tile_pool(name="dram", bufs=1, space="DRAM") as dram,
        ):
            kt_sb = res.tile([P, DC, NK], bf16)
            qt_sb = res.tile([P, DC, NQ], bf16)
            wk1s = res.tile([P, DC, 256], bf16, name="wk1s")
            wk2s = res.tile([P, DC, 256], bf16, name="wk2s")
            wv1s = res.tile([P, DC, 256], bf16, name="wv1s")
            wq1s = res.tile([P, DC, 256], bf16, name="wq1s")
            wq2s = res.tile([P, DC, 256], bf16, name="wq2s")
            wpT = res.tile([P, 2, DIM], bf16)
            nw = res.tile([P, 2], f32)
            pb = res.tile([P, 2], f32)
            lamn = res.tile([1, NH], f32)
            eps_t = res.tile([1, 1], f32)
            ones_l = res.tile([P, 1], f32r)

            q1T = res.tile([P, 2, NQ], bf16)
            k1T = res.tile([P, 2, NK], bf16)
            v1e = res.tile([P, NH, KT, 65], bf16)
            if USE_FP8:
                q2T8 = res.tile([P, 2, NQ], f8)
                k2T8 = res.tile([P, 2, NK], f8)
                v1f8 = res.tile([P, NH, KT, 128], f8)  # 66 used; stride padded
                # packed-hd views for DoubleRow: [32, h, i, n]
                q2f = res.tile([32, NH, 2, NQ], f8)
                k2f = res.tile([32, NH, 2, NK], f8)
            else:
                q2T = res.tile([P, 2, NQ], bf16)
                k2T = res.tile([P, 2, NK], bf16)
            xT = res.tile([P, 2, NQ], f32)

            yb = [dram.tile([G * 257, 512], bf16, name=f"yb{qb}")
                  for qb in range(2)]
            yr = [dram.tile([257, 512], bf16, name=f"yr{qb}")
                  for qb in range(2)]

            # ---- input DMAs, in priority order (SP executes in order) ----
            nc.sync.dma_start(nw[:], nw_d[:])
            nc.sync.dma_start(pb[:], pb_d[:])
            nc.sync.dma_start(lamn[:], lamn_d[:])
            for t_, d_ in ((wk1s, wk1_d), (wq1s, wq1_d)):
                for hh in range(2):
                    nc.sync.dma_start(
                        t_[:, hh * 4:(hh + 1) * 4, :],
                        d_[hh * 512:(hh + 1) * 512, :].rearrange(
                            "(o p) n -> p o n", p=P))
            for d in range(DC):
                nc.sync.dma_start(qt_sb[:, d, 0:512],
                                  qT_d[d * P:(d + 1) * P, 0:512])
            for d in range(DC):
                nc.sync.dma_start(kt_sb[:, d, :], kT_d[d * P:(d + 1) * P, :])
            for t_, d_ in ((wv1s, wv1_d), (wk2s, wk2_d), (wq2s, wq2_d)):
                for hh in range(2):
                    nc.sync.dma_start(
                        t_[:, hh * 4:(hh + 1) * 4, :],
                        d_[hh * 512:(hh + 1) * 512, :].rearrange(
                            "(o p) n -> p o n", p=P))
            for d in range(DC):
                nc.sync.dma_start(qt_sb[:, d, 512:1024],
                                  qT_d[d * P:(d + 1) * P, 512:1024])
            for t in range(2):
                nc.sync.dma_start(wpT[:, t, :], wpT_d[t * P:(t + 1) * P, :])

            # ---- constants (DVE) ----
            nc.vector.memset(eps_t[:], EPS)
            ones_f = res.tile([P, KT], f32)
            nc.vector.memset(ones_f[:], 1.0)
            nc.vector.tensor_copy(ones_l[:], ones_f[:, 0:1])
            zeros_f = res.tile([P, KT], f32)
            nc.vector.memset(zeros_f[:], 0.0)
            for h in range(NH):
                nc.vector.tensor_copy(v1e[:, h, :, 64:65],
                                      ones_f[:].unsqueeze(-1))
                if USE_FP8:
                    nc.vector.tensor_copy(v1f8[:, h, :, 64:65],
                                          ones_f[:].unsqueeze(-1))
                    nc.vector.tensor_copy(v1f8[:, h, :, 65:66],
                                          zeros_f[:].unsqueeze(-1))

            # ---- phase A: q1-proj (qb0) + k1-proj m0 (tracks kT DMA) ----
            with tc.tile_pool(name="ps_a", bufs=6, space="PSUM") as ps_a:
                psq = [ps_a.tile([P, 512], f32, tag="acc", name=f"psq{m}")
                       for m in range(2)]
                psk = [ps_a.tile([P, 512], f32, tag="acc", name=f"psk{kc}")
                       for kc in range(4)]
                for d in range(DC):
                    for m in range(2):
                        nc.tensor.matmul(
                            psq[m][:], wq1s[:, d, m * P:(m + 1) * P],
                            qt_sb[:, d, 0:512],
                            start=(d == 0), stop=(d == DC - 1))
                    for kc in range(4):
                        nc.tensor.matmul(
                            psk[kc][:], wk1s[:, d, 0:P],
                            kt_sb[:, d, kc * 512:(kc + 1) * 512],
                            start=(d == 0), stop=(d == DC - 1))
                for m in range(2):
                    nc.vector.tensor_copy(q1T[:, m, 0:512], psq[m][:])
                for kc in range(4):
                    nc.vector.tensor_copy(
                        k1T[:, 0, kc * 512:(kc + 1) * 512], psk[kc][:])

            if stop_after == "aproj":
                nc.sync.dma_start(y_out_d[0:P, 0:NQ], k1T[:, 0, 0:NQ]
                                  .bitcast(f32)[:, 0:512].unsqueeze(1)
                                  .rearrange("p a n -> p (a n)"))
                return

            # ---- phase B: attention (qb outer, head inner), all other
            #      projection work interleaved as PE filler ----
            xwp_cm = tc.tile_pool(name="xwp", bufs=4)
            x2p_cm = tc.tile_pool(name="x2p", bufs=2)
            xwp = xwp_cm.__enter__()
            x2p = x2p_cm.__enter__()
            xw = {}

            with (
                tc.tile_pool(name="ps_sc", bufs=2, space="PSUM") as ps_sc,
                tc.tile_pool(name="ps_u", bufs=2, space="PSUM") as ps_u,
                tc.tile_pool(name="ps_fil", bufs=2, space="PSUM") as ps_fil,
                tc.tile_pool(name="att", bufs=3) as att,
                tc.tile_pool(name="smal", bufs=2) as smal,
                tc.tile_pool(name="pkp", bufs=3) as pkp,
                tc.tile_pool(name="post", bufs=2) as post,
            ):
                # ---------- filler item machinery ----------
                def proj_items(wsrc, m, dst, n_kc, src_sb, width, dst8=None):
                    """k/q projection accumulations: one psum bank per kc."""
                    for kc in range(n_kc):
                        pst = ps_fil.tile([P, 512], f32, tag="fil",
                                          name=f"pj{m}_{kc}")
                        for d in range(DC):
                            yield lambda pst=pst, d=d, kc=kc: nc.tensor.matmul(
                                pst[:], wsrc[:, d, m * P:(m + 1) * P],
                                src_sb[:, d, kc * 512:(kc + 1) * 512],
                                start=(d == 0), stop=(d == DC - 1))
                        if dst is not None:
                            yield lambda pst=pst, kc=kc: nc.vector.tensor_copy(
                                dst[:, m, kc * 512:(kc + 1) * 512], pst[:])
                        if dst8 is not None:
                            yield lambda pst=pst, kc=kc: nc.vector.tensor_copy(
                                dst8[:, m, kc * 512:(kc + 1) * 512], pst[:])

                def shuffle_items(dst_f, src_8, m, n, qoff=0):
                    """SBUF->SBUF partition regroup [128,m,n] -> [32,h,i,n]."""
                    for hh in range(2):
                        h = m * 2 + hh
                        for i in range(2):
                            yield lambda h=h, i=i, hh=hh: nc.sync.dma_start(
                                dst_f[:, h, i, qoff:qoff + n],
                                src_8[hh * 64 + i * 32:hh * 64 + (i + 1) * 32,
                                      m, qoff:qoff + n])

                def chain(*gens):
                    for g in gens:
                        yield from g

                if USE_FP8:
                    q2dst, q2dst8 = None, q2T8
                    k2dst, k2dst8 = None, k2T8
                else:
                    q2dst, q2dst8 = q2T, None
                    k2dst, k2dst8 = k2T, None

                def v_chunk(kt):
                    psv = ps_fil.tile([P, 256], f32, tag="fil",
                                      name=f"psv{kt}")
                    for d in range(DC):
                        yield lambda psv=psv, d=d, kt=kt: nc.tensor.matmul(
                            psv[:], kt_sb[:, d, kt * P:(kt + 1) * P],
                            wv1s[:, d, :],
                            start=(d == 0), stop=(d == DC - 1))
                    for h in range(NH):
                        yield lambda psv=psv, kt=kt, h=h: \
                            nc.vector.tensor_copy(
                                v1e[:, h, kt, 0:64],
                                psv[:, h * 64:h * 64 + 64])
                    if USE_FP8:
                        for h in range(NH):
                            yield lambda psv=psv, kt=kt, h=h: \
                                nc.vector.tensor_copy(
                                    v1f8[:, h, kt, 0:64],
                                    psv[:, h * 64:h * 64 + 64])

                def lazy(fn, *a, **kw):
                    # defer generator construction (psum tile alloc order)
                    def gen():
                        yield from fn(*a, **kw)
                    return gen()

                fill_chunks = [(f"v{kt}", lazy(v_chunk, kt))
                               for kt in range(KT)]
                fill_chunks.append(
                    ("k2f0", lazy(lambda: chain(
                        proj_items(wk2s, 0, k2dst, 4, kt_sb, NK, dst8=k2dst8),
                        *([shuffle_items(k2f, k2T8, 0, NK)]
                          if USE_FP8 else [])))))
                fill_chunks.append(
                    ("q2f0", lazy(lambda: chain(
                        proj_items(wq2s, 0, q2dst, 1, qt_sb, NQ, dst8=q2dst8),
                        proj_items(wq2s, 1, q2dst, 1, qt_sb, NQ, dst8=q2dst8),
                        *([shuffle_items(q2f, q2T8, 0, 512),
                           shuffle_items(q2f, q2T8, 1, 512)]
                          if USE_FP8 else [])))))
                fill_chunks.append(
                    ("k1m1", lazy(proj_items, wk1s, 1, k1T, 4, kt_sb, NK)))
                fill_chunks.append(
                    ("k2f1", lazy(lambda: chain(
                        proj_items(wk2s, 1, k2dst, 4, kt_sb, NK, dst8=k2dst8),
                        *([shuffle_items(k2f, k2T8, 1, NK)]
                          if USE_FP8 else [])))))
                fc_idx = [0]

                def emit_fill(k):
                    while k > 0 and fc_idx[0] < len(fill_chunks):
                        tag, gen = fill_chunks[fc_idx[0]]
                        it = next(gen, None)
                        if it is None:
                            fc_idx[0] += 1
                            continue
                        it()
                        k -= 1

                def emit_until(tag):
                    # emit every remaining item up to and incl. chunk `tag`
                    idx = next(i for i, (t, _) in enumerate(fill_chunks)
                               if t == tag)
                    while fc_idx[0] <= idx:
                        tag_c, gen = fill_chunks[fc_idx[0]]
                        it = next(gen, None)
                        if it is None:
                            fc_idx[0] += 1
                            continue
                        it()

                def drain_fill():
                    emit_fill(1 << 30)

                def q1_qb1_fix():
                    # q1-proj for qb1 columns: kc index 1 of the 512-col split
                    for m in range(2):
                        pst = ps_fil.tile([P, 512], f32, tag="fil",
                                          name=f"q1b{m}")
                        for d in range(DC):
                            yield lambda pst=pst, d=d, m=m: nc.tensor.matmul(
                                pst[:], wq1s[:, d, m * P:(m + 1) * P],
                                qt_sb[:, d, 512:1024],
                                start=(d == 0), stop=(d == DC - 1))
                        yield lambda pst=pst, m=m: nc.vector.tensor_copy(
                            q1T[:, m, 512:1024], pst[:])

                def q2_qb1_fix():
                    for m in range(2):
                        pst = ps_fil.tile([P, 512], f32, tag="fil",
                                          name=f"q2b{m}")
                        for d in range(DC):
                            yield lambda pst=pst, d=d, m=m: nc.tensor.matmul(
                                pst[:], wq2s[:, d, m * P:(m + 1) * P],
                                qt_sb[:, d, 512:1024],
                                start=(d == 0), stop=(d == DC - 1))
                        if USE_FP8:
                            yield lambda pst=pst, m=m: nc.vector.tensor_copy(
                                q2T8[:, m, 512:1024], pst[:])
                        else:
                            yield lambda pst=pst, m=m: nc.vector.tensor_copy(
                                q2T[:, m, 512:1024], pst[:])
                    if USE_FP8:
                        yield from shuffle_items(q2f, q2T8, 0, 512, qoff=512)
                        yield from shuffle_items(q2f, q2T8, 1, 512, qoff=512)

                fill_chunks.append(("q1b1", lazy(q1_qb1_fix)))
                fill_chunks.append(("q2b1", lazy(q2_qb1_fix)))

                # ---------- per-qb tail: proj partials + ss -> RS ----------
                def emit_tail(qb):
                    qs = slice(qb * 512, (qb + 1) * 512)
                    for t in range(2):
                        xw_t = xwp.tile([P, 512], bf16, tag="xw",
                                        name=f"xw{qb}_{t}")
                        nc.vector.tensor_scalar_mul(xw_t[:], xT[:, t, qs],
                                                    nw[:, t:t + 1])
                        xw[(qb, t)] = xw_t
                    ssps = ps_fil.tile([1, 512], f32, tag="fil",
                                       name=f"ss{qb}")
                    for t in range(2):
                        x2c = x2p.tile([P, 512], f32r, tag="x2")
                        nc.vector.tensor_mul(x2c[:], xT[:, t, qs],
                                             xT[:, t, qs])
                        nc.tensor.matmul(ssps[0:1, :], ones_l[:], x2c[:],
                                         start=(t == 0), stop=(t == 1))
                    ss_sb = pkp.tile([1, 512], bf16, tag="sssb")
                    nc.vector.tensor_copy(ss_sb[:], ssps[:])
                    for gb in range(G):
                        nc.sync.dma_start(
                            yb[qb][gb * 257 + 256:gb * 257 + 257, :],
                            ss_sb[:])
                    for m in range(8):
                        yp = ps_fil.tile([P, 512], f32, tag="fil",
                                         name=f"yp{qb}_{m}")
                        for t in range(2):
                            nc.tensor.matmul(
                                yp[:], wpT[:, t, m * P:(m + 1) * P],
                                xw[(qb, t)][:], start=(t == 0), stop=(t == 1))
                        yp_sb = pkp.tile([P, 512], bf16, tag="ypsb")
                        nc.vector.tensor_copy(yp_sb[:], yp[:])
                        row0 = (m // 2) * 257 + (m % 2) * P
                        nc.sync.dma_start(yb[qb][row0:row0 + P, :], yp_sb[:])

                def emit_rs(qb):
                    nc.gpsimd.collective_compute(
                        "ReduceScatter",
                        mybir.AluOpType.add,
                        replica_groups=GROUPS,
                        ins=[yb[qb].opt()],
                        outs=[yr[qb].opt()],
                    )

                def emit_post(qb):
                    qs = slice(qb * 512, (qb + 1) * 512)
                    ss_row = post.tile([1, 512], bf16, tag="ssrow")
                    nc.sync.dma_start(ss_row[:], yr[qb][256:257, :])
                    s_row = post.tile([1, 512], f32, tag="srow")
                    nc.scalar.activation(s_row[:], ss_row[:], AF.Sqrt,
                                         bias=eps_t[0:1, 0:1], scale=1.0 / DIM)
                    nc.vector.reciprocal(s_row[:], s_row[:])
                    s_b = post.tile([P, 512], f32, tag="sb")
                    nc.gpsimd.partition_broadcast(s_b[:], s_row[:])
                    for t in range(2):
                        yred_t = post.tile([P, 512], bf16, tag="yred")
                        nc.sync.dma_start(yred_t[:], yr[qb][t * P:(t + 1) * P, :])
                        yo = post.tile([P, 512], f32, tag="yo")
                        nc.vector.tensor_mul(yo[:], yred_t[:], s_b[:])
                        nc.vector.tensor_scalar_add(yo[:], yo[:],
                                                    pb[:, t:t + 1])
                        nc.sync.dma_start(y_out_d[t * P:(t + 1) * P, qs],
                                          yo[:])

                # ---------- attention ----------
                for qb in range(2):
                    qs = slice(qb * 512, (qb + 1) * 512)
                    for h in range(NH):
                        po = (h % 2) * 64
                        mi = h // 2
                        if mi == 1:
                            emit_until("k1m1")
                        if qb == 1:
                            emit_until("q1b1")
                        u_ps = []
                        # branch 1 (bf16)
                        u1 = ps_u.tile([66, 512], f32, tag="u")
                        u_ps.append(u1)
                        for kg in range(KT // 2):
                            emit_until(f"v{kg * 2 + 1}")
                            sc = ps_sc.tile([P, 1024], f32, tag="sc")
                            for j in range(2):
                                kt = kg * 2 + j
                                nc.tensor.matmul(
                                    sc[:, j * 512:(j + 1) * 512],
                                    k1T[po:po + 64, mi, kt * P:(kt + 1) * P],
                                    q1T[po:po + 64, mi, qs],
                                    start=True, stop=True)
                            emit_fill(1)
                            e_t = att.tile([P, 1024], bf16, tag="e")
                            nc.scalar.activation(e_t[:], sc[:], AF.Exp,
                                                 scale=SCALE)
                            for j in range(2):
                                kt = kg * 2 + j
                                nc.tensor.matmul(
                                    u1[0:65, :], v1e[:, h, kt, :],
                                    e_t[:, j * 512:(j + 1) * 512],
                                    start=(kt == 0), stop=(kt == KT - 1))
                            emit_fill(2)
                        # branch 2 (fp8 DoubleRow or bf16 fallback)
                        if mi == 0:
                            emit_until("q2f0")
                        else:
                            emit_until("k2f1")
                        if qb == 1:
                            emit_until("q2b1")
                        u2 = ps_u.tile([66, 512], f32, tag="u")
                        u_ps.append(u2)
                        for kg in range(KT // 2):
                            sc = ps_sc.tile([P, 1024], f32, tag="sc")
                            if USE_FP8:
                                for j in range(2):
                                    kt = kg * 2 + j
                                    nc.tensor.matmul(
                                        sc[:, j * 512:(j + 1) * 512],
                                        k2f[:, h, :, kt * P:(kt + 1) * P],
                                        q2f[:, h, :, qs],
                                        start=True, stop=True, perf_mode=DR)
                                emit_fill(2)
                                e_t = att.tile([P, 2, 512], f8, tag="e8")
                                nc.scalar.activation(
                                    e_t[:].rearrange("p a b -> p (a b)"),
                                    sc[:], AF.Exp, scale=SCALE)
                                nc.tensor.matmul(
                                    u2[:],
                                    v1f8[:, h, kg * 2:kg * 2 + 2, 0:66],
                                    e_t[:],
                                    start=(kg == 0), stop=(kg == KT // 2 - 1),
                                    perf_mode=DR)
                                emit_fill(2)
                            else:
                                for j in range(2):
                                    kt = kg * 2 + j
                                    nc.tensor.matmul(
                                        sc[:, j * 512:(j + 1) * 512],
                                        k2T[po:po + 64, mi, kt * P:(kt + 1) * P],
                                        q2T[po:po + 64, mi, qs],
                                        start=True, stop=True)
                                emit_fill(1)
                                e_t = att.tile([P, 1024], bf16, tag="e")
                                nc.scalar.activation(e_t[:], sc[:], AF.Exp,
                                                     scale=SCALE)
                                for j in range(2):
                                    kt = kg * 2 + j
                                    nc.tensor.matmul(
                                        u2[0:65, :], v1e[:, h, kt, :],
                                        e_t[:, j * 512:(j + 1) * 512],
                                        start=(kt == 0), stop=(kt == KT - 1))
                                emit_fill(2)
                        # combine: x = u1/l1 - lambda*u2/l2
                        rr1 = smal.tile([1, 512], f32, tag="rr")
                        nc.vector.reciprocal(rr1[:], u_ps[0][64:65, :])
                        rr2 = smal.tile([1, 512], f32, tag="rr")
                        nc.vector.reciprocal(rr2[:], u_ps[1][64:65, :])
                        nc.vector.tensor_scalar_mul(rr2[:], rr2[:],
                                                    lamn[0:1, h:h + 1])
                        rr1b = smal.tile([64, 512], f32, tag="rrb")
                        nc.gpsimd.partition_broadcast(rr1b[:], rr1[:])
                        rr2b = smal.tile([64, 512], f32, tag="rrb")
                        nc.gpsimd.partition_broadcast(rr2b[:], rr2[:])
                        t1 = smal.tile([64, 512], f32, tag="tt")
                        nc.vector.tensor_mul(t1[:], u_ps[0][0:64, :], rr1b[:])
                        t2 = smal.tile([64, 512], f32, tag="tt")
                        nc.vector.tensor_mul(t2[:], u_ps[1][0:64, :], rr2b[:])
                        nc.vector.tensor_add(xT[po:po + 64, mi, qs],
                                             t1[:], t2[:])

                    # end of q-block: emit its projection tail; qb0's
                    # ReduceScatter + post hide under qb1's attention
                    if qb == 0:
                        fill_chunks.append(("tail0", iter([
                            lambda: emit_tail(0),
                            lambda: emit_rs(0),
                            lambda: emit_post(0),
                        ])))
                    else:
                        drain_fill()
                        emit_tail(1)
                        if stop_after == "precc":
                            return
                        emit_rs(1)
                        emit_post(1)

                if stop_after == "attn":
                    pass

            x2p_cm.__exit__(None, None, None)
            xwp_cm.__exit__(None, None, None)

    with tile.TileContext(nc) as tc:
        if loop_n:
            with tc.For_i(0, loop_n, 1):
                _trace(tc)
        else:
            _trace(tc)
    nc.compile()
    return nc


_CACHE = {}


def _get_nc():
    if "nc" not in _CACHE:
        _CACHE["nc"] = _build()
    return _CACHE["nc"]


def _shard_inputs(inputs):
    q = np.asarray(inputs["query"], np.float32)
    k = np.asarray(inputs["key"], np.float32)
    q1_w = np.asarray(inputs["q1_w"], np.float32)
    q2_w = np.asarray(inputs["q2_w"], np.float32)
    kv1_w = np.asarray(inputs["kv1_w"], np.float32)
    kv2_w = np.asarray(inputs["kv2_w"], np.float32)
    proj_w = np.asarray(inputs["proj_w"], np.float32)
    proj_b = np.asarray(inputs["proj_b"], np.float32)
    norm_w = np.asarray(inputs["norm_w"], np.float32)
    lam1 = np.asarray(inputs["lambda_1"], np.float32).reshape(H)
    lam2 = np.asarray(inputs["lambda_2"], np.float32).reshape(H)
    lam_full = lam1 - lam2 + LAMBDA_INIT

    from ml_dtypes import bfloat16

    def c(a):
        return np.ascontiguousarray(a.astype(bfloat16))

    cf = np.ascontiguousarray
    in_maps = []
    for r in range(8):
        b, g = r // G, r % G
        rows = slice(g * 256, (g + 1) * 256)
        vrows = slice(DIM + g * 256, DIM + (g + 1) * 256)
        in_maps.append({
            "qT": c(q[b].T),
            "kT": c(k[b].T),
            "wq1": c(q1_w[rows].T),
            "wq2": c(q2_w[rows].T),
            "wk1": c(kv1_w[rows].T),
            "wv1": c(kv1_w[vrows].T),
            "wk2": c(kv2_w[rows].T),
            "wpT": c(proj_w[:, rows].T),
            "nw": cf(norm_w[rows].reshape(2, P).T),
            "pb": cf(proj_b[rows].reshape(2, P).T),
            "lamn": cf(-lam_full[g * NH:(g + 1) * NH].reshape(1, NH)),
        })
    return in_maps


def kernel(**inputs):
    from concourse.bass_utils import run_bass_kernel_spmd

    nc = _get_nc()
    in_maps = _shard_inputs(inputs)
    res = run_bass_kernel_spmd(nc, in_maps, core_ids=list(range(8)))
    out = np.empty((B, NQ, DIM), np.float32)
    for r in range(8):
        b, g = r // G, r % G
        out[b, :, g * 256:(g + 1) * 256] = res.results[r]["y_out"].T
    return out
